# revision 5
# baseline (speedup 1.0000x reference)
"""Single-head attention on 8 Trainium2 NeuronCores, batch-sharded.

Per core (one batch element b):
  x_b        [S=2048, D=768]  (fed pre-transposed as xT [768, 2048])
  q/k/v^T    [64, 2048] = W^T @ x^T        (PE, fp32r)
  scores^T   [k=128-tile, q-chunk=512] = (K^T tile)^T-contraction Q^T (PE)
  P^T        = exp(scores^T / 8)           (ACT; no max-subtraction needed:
                                            |scores/8| <~ 2, exp is safe)
  out^T,den  = [V | 1]^T-accumulation P^T  (PE; ones column gives the
                                            softmax denominator row)
  out        = transpose(out^T) rows / den (PE transpose + DVE normalize)

All heavy matmuls run as fp32r (1 cycle/row at N=512). No inter-core
communication: batch is fully data-parallel.
"""

import numpy as np

B, S, D, H = 8, 2048, 768, 64
DT = D // 128  # 6 d-tiles
NQ = S // 512  # 4 q-chunks of 512
NK = S // 128  # 16 k-tiles of 128
SCALE = 1.0 / np.sqrt(H).item()

_cache = {}


def _build():
    import concourse.mybir as mybir
    import concourse.tile as tile
    from concourse import bacc
    from concourse.masks import make_identity

    f32 = mybir.dt.float32
    f32r = mybir.dt.float32r
    Exp = mybir.ActivationFunctionType.Exp

    nc = bacc.Bacc(None)
    xT_d = nc.dram_tensor("xT", [D, S], f32, kind="ExternalInput")
    wq_d = nc.dram_tensor("wq", [D, H], f32, kind="ExternalInput")
    wk_d = nc.dram_tensor("wk", [D, H], f32, kind="ExternalInput")
    wv_d = nc.dram_tensor("wv", [D, H], f32, kind="ExternalInput")
    bq_d = nc.dram_tensor("bq", [H, 1], f32, kind="ExternalInput")
    bk_d = nc.dram_tensor("bk", [H, 1], f32, kind="ExternalInput")
    bv_d = nc.dram_tensor("bv", [H, 1], f32, kind="ExternalInput")
    ones_d = nc.dram_tensor("ones", [128, NK], f32, kind="ExternalInput")
    out_d = nc.dram_tensor("out", [S, H], f32, kind="ExternalOutput")

    with tile.TileContext(nc) as tc:
        with (
            tc.tile_pool(name="big", bufs=1) as big,
            tc.tile_pool(name="small", bufs=1) as small,
            tc.tile_pool(name="pt", bufs=3) as ptp,
            tc.tile_pool(name="res", bufs=3) as resp,
            tc.tile_pool(name="psA", bufs=3, space="PSUM") as psA,
            tc.tile_pool(name="psO", bufs=1, space="PSUM") as psO,
            tc.tile_pool(name="psT", bufs=1, space="PSUM") as psT,
        ):
            # ---- constants / weights ----
            ident = small.tile([128, 128], f32)
            make_identity(nc, ident)
            w_sb = {}
            b_sb = {}
            for name, wd, bd in (
                ("q", wq_d, bq_d),
                ("k", wk_d, bk_d),
                ("v", wv_d, bv_d),
            ):
                wt = small.tile([128, DT, H], f32r, tag=f"w{name}")
                nc.sync.dma_start(
                    out=wt,
                    in_=wd[:, :].rearrange("(t p) h -> p t h", p=128).bitcast(f32r),
                )
                bt = small.tile([H, 1], f32, tag=f"b{name}")
                nc.sync.dma_start(out=bt, in_=bd[:, :])
                w_sb[name] = wt
                b_sb[name] = bt

            # ---- x^T ----
            xT = big.tile([128, DT, S], f32r)
            for dt in range(DT):
                nc.sync.dma_start(
                    out=xT[:, dt, :],
                    in_=xT_d[dt * 128 : (dt + 1) * 128, :].bitcast(f32r),
                )

            # ---- QKV projections: {q,k,v}^T [64, 2048] ----
            proj = {}
            for name in ("q", "k", "v"):
                pT = big.tile([H, S], f32r if name != "v" else f32, tag=f"{name}T")
                for qc in range(NQ):
                    ps = psA.tile([H, 512], f32, tag="sc")
                    for dt in range(DT):
                        nc.tensor.matmul(
                            ps,
                            lhsT=w_sb[name][:, dt, :],
                            rhs=xT[:, dt, qc * 512 : (qc + 1) * 512],
                            start=(dt == 0),
                            stop=(dt == DT - 1),
                        )
                    nc.vector.tensor_scalar_add(
                        out=pT[:, qc * 512 : (qc + 1) * 512],
                        in0=ps,
                        scalar1=b_sb[name],
                    )
                proj[name] = pT
            qT, kT, vT = proj["q"], proj["k"], proj["v"]

            # ---- V natural layout [k, h] with ones column: [128, 65] per k-tile
            v65 = big.tile([128, NK, H + 1], f32r)
            nc.sync.dma_start(
                out=v65[:, :, H : H + 1],
                in_=ones_d[:, :].unsqueeze(2).bitcast(f32r),
            )
            for kt in range(NK):
                tp = psT.tile([128, H], f32, tag="tr")
                nc.tensor.transpose(
                    tp, vT[:, kt * 128 : (kt + 1) * 128], ident[:H, :H]
                )
                nc.vector.tensor_copy(out=v65[:, kt, :H], in_=tp)

            # ---- main loop: scores^T -> exp -> PV accumulation ----
            outps = [
                psO.tile([H + 1, 512], f32, tag=f"o{qc}", name=f"outps{qc}")
                for qc in range(NQ)
            ]
            for kt in range(NK):
                pT = ptp.tile([128, S], f32r)
                for qc in range(NQ):
                    sps = psA.tile([128, 512], f32, tag="sc")
                    nc.tensor.matmul(
                        sps,
                        lhsT=kT[:, kt * 128 : (kt + 1) * 128],
                        rhs=qT[:, qc * 512 : (qc + 1) * 512],
                        start=True,
                        stop=True,
                    )
                    nc.scalar.activation(
                        out=pT[:, qc * 512 : (qc + 1) * 512],
                        in_=sps,
                        func=Exp,
                        scale=SCALE,
                    )
                for qc in range(NQ):
                    nc.tensor.matmul(
                        outps[qc],
                        lhsT=v65[:, kt, :],
                        rhs=pT[:, qc * 512 : (qc + 1) * 512],
                        start=(kt == 0),
                        stop=(kt == NK - 1),
                    )

            # ---- finalize: transpose out^T back, divide by denominator ----
            oT = big.tile([H + 1, S], f32)
            for qc in range(NQ):
                nc.vector.tensor_copy(
                    out=oT[:, qc * 512 : (qc + 1) * 512], in_=outps[qc]
                )
            for st in range(NK):
                tp = psT.tile([128, H + 1], f32, tag="tr")
                nc.tensor.transpose(
                    tp, oT[:, st * 128 : (st + 1) * 128], ident[: H + 1, : H + 1]
                )
                rec = resp.tile([128, 1], f32, tag="rec")
                nc.vector.reciprocal(out=rec, in_=tp[:, H : H + 1])
                res = resp.tile([128, H], f32, tag="res")
                nc.vector.tensor_scalar_mul(out=res, in0=tp[:, :H], scalar1=rec)
                nc.sync.dma_start(out=out_d[st * 128 : (st + 1) * 128, :], in_=res)

    nc.compile()
    return nc


def _get_nc():
    if "nc" not in _cache:
        _cache["nc"] = _build()
    return _cache["nc"]


def kernel(x, Wq, bq, Wk, bk, Wv, bv, **_):
    from concourse.bass_utils import run_bass_kernel_spmd

    nc = _get_nc()
    x = np.asarray(x, dtype=np.float32)
    common = {
        "wq": np.ascontiguousarray(np.asarray(Wq, np.float32)),
        "wk": np.ascontiguousarray(np.asarray(Wk, np.float32)),
        "wv": np.ascontiguousarray(np.asarray(Wv, np.float32)),
        "bq": np.ascontiguousarray(np.asarray(bq, np.float32).reshape(H, 1)),
        "bk": np.ascontiguousarray(np.asarray(bk, np.float32).reshape(H, 1)),
        "bv": np.ascontiguousarray(np.asarray(bv, np.float32).reshape(H, 1)),
        "ones": np.ones((128, NK), np.float32),
    }
    in_maps = [
        {"xT": np.ascontiguousarray(x[b].T), **common} for b in range(B)
    ]
    res = run_bass_kernel_spmd(nc, in_maps, core_ids=list(range(B)))
    return np.stack([res.results[b]["out"] for b in range(B)])


# revision 8
# speedup vs baseline: 1.1469x; 1.1469x over previous
"""Single-head attention on 8 Trainium2 NeuronCores, batch-sharded.

Per core (one batch element b):
  x_b        [S=2048, D=768]  (fed pre-transposed as xT [768, 2048])
  q/k/v^T    [64, 2048] = W^T @ x^T        (PE, fp32r; q/k stored bf16)
  scores^T   [k-tile=128, q-chunk] = K^T-tile x Q^T   (PE, bf16)
  P^T        = exp(scores^T / 8)           (ACT -> bf16; no max-subtraction:
                                            |scores/8| <~ 2, exp is safe)
  out^T,den  = [V | 1] x P^T accumulation  (PE, bf16; ones column gives the
                                            softmax denominator row)
  out        = transpose(out^T) rows / den (PE transpose + DVE normalize)

The PV stage is software-pipelined one k-tile behind the scores stage so
the PE never blocks on ACT exp. No inter-core communication.
"""

import numpy as np

B, S, D, H = 8, 2048, 768, 64
DT = D // 128  # 6 d-tiles
NQ = S // 512  # 4 q-chunks of 512
NC2 = S // 1024  # 2 exp chunks of 1024
NK = S // 128  # 16 k-tiles of 128
SCALE = 1.0 / np.sqrt(H).item()

_cache = {}


def _build():
    import concourse.mybir as mybir
    import concourse.tile as tile
    from concourse import bacc
    from concourse.masks import make_identity

    f32 = mybir.dt.float32
    f32r = mybir.dt.float32r
    bf16 = mybir.dt.bfloat16
    Exp = mybir.ActivationFunctionType.Exp

    nc = bacc.Bacc(None)
    xT_d = nc.dram_tensor("xT", [D, S], f32, kind="ExternalInput")
    wq_d = nc.dram_tensor("wq", [D, H], f32, kind="ExternalInput")
    wk_d = nc.dram_tensor("wk", [D, H], f32, kind="ExternalInput")
    wv_d = nc.dram_tensor("wv", [D, H], f32, kind="ExternalInput")
    bq_d = nc.dram_tensor("bq", [H, 1], f32, kind="ExternalInput")
    bk_d = nc.dram_tensor("bk", [H, 1], f32, kind="ExternalInput")
    bv_d = nc.dram_tensor("bv", [H, 1], f32, kind="ExternalInput")
    ones_d = nc.dram_tensor("ones", [128, NK], bf16, kind="ExternalInput")
    out_d = nc.dram_tensor("out", [S, H], f32, kind="ExternalOutput")

    with tile.TileContext(nc) as tc:
        with (
            tc.tile_pool(name="big", bufs=1) as big,
            tc.tile_pool(name="small", bufs=1) as small,
            tc.tile_pool(name="pt", bufs=3) as ptp,
            tc.tile_pool(name="res", bufs=3) as resp,
            tc.tile_pool(name="psA", bufs=2, space="PSUM") as psA,
            tc.tile_pool(name="psO", bufs=1, space="PSUM") as psO,
        ):
            # ---- constants / weights ----
            ident = small.tile([128, 128], f32)
            make_identity(nc, ident)
            identb = small.tile([128, 128], bf16)
            nc.gpsimd.tensor_copy(out=identb, in_=ident)
            w_sb = {}
            b_sb = {}
            for name, wd, bd in (
                ("q", wq_d, bq_d),
                ("k", wk_d, bk_d),
                ("v", wv_d, bv_d),
            ):
                wt = small.tile([128, DT, H], f32r, tag=f"w{name}")
                nc.sync.dma_start(
                    out=wt,
                    in_=wd[:, :].rearrange("(t p) h -> p t h", p=128).bitcast(f32r),
                )
                bt = small.tile([H, 1], f32, tag=f"b{name}")
                nc.sync.dma_start(out=bt, in_=bd[:, :])
                w_sb[name] = wt
                b_sb[name] = bt

            # ---- x^T ----
            xT = big.tile([128, DT, S], f32r)
            for dt in range(DT):
                nc.sync.dma_start(
                    out=xT[:, dt, :],
                    in_=xT_d[dt * 128 : (dt + 1) * 128, :].bitcast(f32r),
                )

            # ---- QKV projections: q/k bf16 [64, 2048], v f32 ----
            proj = {}
            for name in ("q", "k", "v"):
                pj = big.tile(
                    [H, S], f32 if name == "v" else bf16, tag=f"{name}T"
                )
                for qc in range(NQ):
                    ps = psA.tile([H, 512], f32, tag="a")
                    for dt in range(DT):
                        nc.tensor.matmul(
                            ps,
                            lhsT=w_sb[name][:, dt, :],
                            rhs=xT[:, dt, qc * 512 : (qc + 1) * 512],
                            start=(dt == 0),
                            stop=(dt == DT - 1),
                        )
                    nc.vector.tensor_scalar_add(
                        out=pj[:, qc * 512 : (qc + 1) * 512],
                        in0=ps,
                        scalar1=b_sb[name],
                    )
                proj[name] = pj
            qT, kT, vT = proj["q"], proj["k"], proj["v"]

            # ---- V natural layout [k, h] + ones column: [128, 65] bf16 ----
            v65 = big.tile([128, NK, H + 1], bf16)
            nc.sync.dma_start(out=v65[:, :, H : H + 1], in_=ones_d[:, :].unsqueeze(2))
            for kt in range(NK):
                tp = psA.tile([128, H], f32, tag="a", name="vtr")
                nc.tensor.transpose(
                    tp, vT[:, kt * 128 : (kt + 1) * 128], ident[:H, :H]
                )
                nc.vector.tensor_copy(out=v65[:, kt, :H], in_=tp)

            # ---- main loop: scores^T -> exp -> PV (PV pipelined 1 kt behind)
            outps = [
                psO.tile([H + 1, 512], f32, tag=f"o{qc}", name=f"outps{qc}")
                for qc in range(NQ)
            ]
            pTs = [None] * NK
            for kt in range(NK + 1):
                if kt < NK:
                    pT = ptp.tile([128, S], bf16, tag="pT", name=f"pT{kt}")
                    pTs[kt] = pT
                    for c2 in range(NC2):
                        sc = psA.tile([128, 1024], f32, tag="a", name=f"sc{kt}_{c2}")
                        for h2 in range(2):
                            qc = c2 * 2 + h2
                            nc.tensor.matmul(
                                sc[:, h2 * 512 : (h2 + 1) * 512],
                                lhsT=kT[:, kt * 128 : (kt + 1) * 128],
                                rhs=qT[:, qc * 512 : (qc + 1) * 512],
                                start=True,
                                stop=True,
                            )
                        nc.scalar.activation(
                            out=pT[:, c2 * 1024 : (c2 + 1) * 1024],
                            in_=sc,
                            func=Exp,
                            scale=SCALE,
                        )
                if kt >= 1:
                    pk = kt - 1
                    for qc in range(NQ):
                        nc.tensor.matmul(
                            outps[qc],
                            lhsT=v65[:, pk, :],
                            rhs=pTs[pk][:, qc * 512 : (qc + 1) * 512],
                            start=(pk == 0),
                            stop=(pk == NK - 1),
                        )

            # ---- finalize: transpose out^T back, divide by denominator ----
            oT = big.tile([H + 1, S], bf16)
            for qc in range(NQ):
                nc.vector.tensor_copy(
                    out=oT[:, qc * 512 : (qc + 1) * 512], in_=outps[qc]
                )
            for st in range(NK):
                tp = psA.tile([128, H + 1], bf16, tag="a", name="otr")
                nc.tensor.transpose(
                    tp, oT[:, st * 128 : (st + 1) * 128], identb[: H + 1, : H + 1]
                )
                rec = resp.tile([128, 1], f32, tag="rec")
                nc.vector.reciprocal(out=rec, in_=tp[:, H : H + 1])
                res = resp.tile([128, H], f32, tag="res")
                nc.vector.tensor_scalar_mul(out=res, in0=tp[:, :H], scalar1=rec)
                nc.sync.dma_start(out=out_d[st * 128 : (st + 1) * 128, :], in_=res)

    nc.compile()
    return nc


def _get_nc():
    if "nc" not in _cache:
        _cache["nc"] = _build()
    return _cache["nc"]


def _ones_bf16():
    import ml_dtypes

    return np.ones((128, NK), ml_dtypes.bfloat16)


def kernel(x, Wq, bq, Wk, bk, Wv, bv, **_):
    from concourse.bass_utils import run_bass_kernel_spmd

    nc = _get_nc()
    x = np.asarray(x, dtype=np.float32)
    common = {
        "wq": np.ascontiguousarray(np.asarray(Wq, np.float32)),
        "wk": np.ascontiguousarray(np.asarray(Wk, np.float32)),
        "wv": np.ascontiguousarray(np.asarray(Wv, np.float32)),
        "bq": np.ascontiguousarray(np.asarray(bq, np.float32).reshape(H, 1)),
        "bk": np.ascontiguousarray(np.asarray(bk, np.float32).reshape(H, 1)),
        "bv": np.ascontiguousarray(np.asarray(bv, np.float32).reshape(H, 1)),
        "ones": _ones_bf16(),
    }
    in_maps = [
        {"xT": np.ascontiguousarray(x[b].T), **common} for b in range(B)
    ]
    res = run_bass_kernel_spmd(nc, in_maps, core_ids=list(range(B)))
    return np.stack([res.results[b]["out"] for b in range(B)])


# revision 9
# speedup vs baseline: 1.5471x; 1.3489x over previous
"""Single-head attention on 8 Trainium2 NeuronCores, batch-sharded.

Per core (one batch element b):
  x_b        [S=2048, D=768]  (fed pre-transposed as xT [768, 2048])
  q/k/v^T    [64, 2048] = W^T @ x^T        (PE, fp32r; q/k stored bf16 in
                                            rows 0-63 of a zero-padded
                                            [128, 2048] tile: the K=128
                                            contraction streams 2x faster
                                            than K=64 on TRN2)
  scores^T   [k-tile=128, q-chunk] = K^T-tile x Q^T   (PE, bf16, K=128)
  P^T        = exp(scores^T / 8)           (ACT -> bf16; no max-subtraction:
                                            |scores/8| <~ 2, exp is safe)
  out^T,den  = [V | 1] x P^T accumulation  (PE, bf16; ones column gives the
                                            softmax denominator row)
  out        = transpose(out^T) rows / den (PE transpose + DVE normalize)

x^T is DMA'd in s-column chunks and the QKV projections are emitted
chunk-interleaved so the PE starts ~4us in instead of waiting for the
whole 6MB. The PV stage is software-pipelined one k-tile behind the
scores stage, and V-transposes ride inside the main loop, so the PE
stays dense (HAM stays un-throttled). No inter-core communication.
"""

import numpy as np

B, S, D, H = 8, 2048, 768, 64
DT = D // 128  # 6 d-tiles
NQ = S // 512  # 4 q-chunks of 512
NC2 = S // 1024  # 2 exp chunks of 1024
NK = S // 128  # 16 k-tiles of 128
SCALE = 1.0 / np.sqrt(H).item()

_cache = {}


def _build():
    import concourse.mybir as mybir
    import concourse.tile as tile
    from concourse import bacc
    from concourse.masks import make_identity

    f32 = mybir.dt.float32
    f32r = mybir.dt.float32r
    bf16 = mybir.dt.bfloat16
    Exp = mybir.ActivationFunctionType.Exp

    nc = bacc.Bacc(None)
    xT_d = nc.dram_tensor("xT", [D, S], f32, kind="ExternalInput")
    wq_d = nc.dram_tensor("wq", [D, H], f32, kind="ExternalInput")
    wk_d = nc.dram_tensor("wk", [D, H], f32, kind="ExternalInput")
    wv_d = nc.dram_tensor("wv", [D, H], f32, kind="ExternalInput")
    bq_d = nc.dram_tensor("bq", [H, 1], f32, kind="ExternalInput")
    bk_d = nc.dram_tensor("bk", [H, 1], f32, kind="ExternalInput")
    bv_d = nc.dram_tensor("bv", [H, 1], f32, kind="ExternalInput")
    ones_d = nc.dram_tensor("ones", [128, NK], bf16, kind="ExternalInput")
    out_d = nc.dram_tensor("out", [S, H], f32, kind="ExternalOutput")

    with tile.TileContext(nc) as tc:
        with (
            tc.tile_pool(name="big", bufs=1) as big,
            tc.tile_pool(name="small", bufs=1) as small,
            tc.tile_pool(name="pt", bufs=3) as ptp,
            tc.tile_pool(name="res", bufs=3) as resp,
            tc.tile_pool(name="psA", bufs=2, space="PSUM") as psA,
            tc.tile_pool(name="psO", bufs=1, space="PSUM") as psO,
        ):
            # ---- constants / weights ----
            ident = small.tile([128, 128], f32)
            make_identity(nc, ident)
            identb = small.tile([128, 128], bf16)
            nc.gpsimd.tensor_copy(out=identb, in_=ident)
            w_sb = {}
            b_sb = {}
            for name, wd, bd in (
                ("q", wq_d, bq_d),
                ("k", wk_d, bk_d),
                ("v", wv_d, bv_d),
            ):
                wt = small.tile([128, DT, H], f32r, tag=f"w{name}")
                nc.sync.dma_start(
                    out=wt,
                    in_=wd[:, :].rearrange("(t p) h -> p t h", p=128).bitcast(f32r),
                )
                bt = small.tile([H, 1], f32, tag=f"b{name}")
                nc.sync.dma_start(out=bt, in_=bd[:, :])
                w_sb[name] = wt
                b_sb[name] = bt

            # ---- x^T, DMA'd in s-column chunks so early chunks arrive fast
            xT = big.tile([128, DT, S], f32r)
            for qc in range(NQ):
                for dt in range(DT):
                    nc.sync.dma_start(
                        out=xT[:, dt, qc * 512 : (qc + 1) * 512],
                        in_=xT_d[
                            dt * 128 : (dt + 1) * 128, qc * 512 : (qc + 1) * 512
                        ].bitcast(f32r),
                    )

            # ---- QKV projections, chunk-interleaved ----
            # q/k: [128, 2048] bf16, data in rows 0-63, rows 64-127 zeroed
            # (K=128 matmuls stream 2x faster than K=64 on TRN2).
            qT = big.tile([128, S], bf16, tag="qT")
            kT = big.tile([128, S], bf16, tag="kT")
            vT = big.tile([H, S], f32, tag="vT")
            nc.gpsimd.memset(qT[H:128, :], 0.0)
            nc.gpsimd.memset(kT[H:128, :], 0.0)
            proj = {"q": qT, "k": kT, "v": vT}
            for qc in range(NQ):
                for name in ("q", "k", "v"):
                    ps = psA.tile([H, 512], f32, tag="a", name=f"ps_{name}{qc}")
                    for dt in range(DT):
                        nc.tensor.matmul(
                            ps,
                            lhsT=w_sb[name][:, dt, :],
                            rhs=xT[:, dt, qc * 512 : (qc + 1) * 512],
                            start=(dt == 0),
                            stop=(dt == DT - 1),
                        )
                    nc.vector.tensor_scalar_add(
                        out=proj[name][:H, qc * 512 : (qc + 1) * 512],
                        in0=ps,
                        scalar1=b_sb[name],
                    )

            # ---- V layout [k, h] + ones column (bf16), filled in main loop
            v65 = big.tile([128, NK, H + 1], bf16)
            nc.sync.dma_start(out=v65[:, :, H : H + 1], in_=ones_d[:, :].unsqueeze(2))

            # ---- main loop: vtrans + scores^T -> exp -> PV (PV one kt behind)
            outps = [
                psO.tile([H + 1, 512], f32, tag=f"o{qc}", name=f"outps{qc}")
                for qc in range(NQ)
            ]
            pTs = [None] * NK
            for kt in range(NK + 1):
                if kt < NK:
                    # V-transpose for this k-tile (PE, cheap, keeps PE dense)
                    tp = psA.tile([128, H], f32, tag="a", name=f"vtr{kt}")
                    nc.tensor.transpose(
                        tp, vT[:, kt * 128 : (kt + 1) * 128], ident[:H, :H]
                    )
                    nc.vector.tensor_copy(out=v65[:, kt, :H], in_=tp)
                    pT = ptp.tile([128, S], bf16, tag="pT", name=f"pT{kt}")
                    pTs[kt] = pT
                    for c2 in range(NC2):
                        sc = psA.tile([128, 1024], f32, tag="a", name=f"sc{kt}_{c2}")
                        for h2 in range(2):
                            qc = c2 * 2 + h2
                            nc.tensor.matmul(
                                sc[:, h2 * 512 : (h2 + 1) * 512],
                                lhsT=kT[:, kt * 128 : (kt + 1) * 128],
                                rhs=qT[:, qc * 512 : (qc + 1) * 512],
                                start=True,
                                stop=True,
                            )
                        nc.scalar.activation(
                            out=pT[:, c2 * 1024 : (c2 + 1) * 1024],
                            in_=sc,
                            func=Exp,
                            scale=SCALE,
                        )
                if kt >= 1:
                    pk = kt - 1
                    for qc in range(NQ):
                        nc.tensor.matmul(
                            outps[qc],
                            lhsT=v65[:, pk, :],
                            rhs=pTs[pk][:, qc * 512 : (qc + 1) * 512],
                            start=(pk == 0),
                            stop=(pk == NK - 1),
                        )

            # ---- finalize: transpose out^T back, divide by denominator ----
            oT = big.tile([H + 1, S], bf16)
            for qc in range(NQ):
                nc.vector.tensor_copy(
                    out=oT[:, qc * 512 : (qc + 1) * 512], in_=outps[qc]
                )
            for st in range(NK):
                tp = psA.tile([128, H + 1], bf16, tag="a", name=f"otr{st}")
                nc.tensor.transpose(
                    tp, oT[:, st * 128 : (st + 1) * 128], identb[: H + 1, : H + 1]
                )
                rec = resp.tile([128, 1], f32, tag="rec")
                nc.vector.reciprocal(out=rec, in_=tp[:, H : H + 1])
                res = resp.tile([128, H], f32, tag="res")
                nc.vector.tensor_scalar_mul(out=res, in0=tp[:, :H], scalar1=rec)
                nc.sync.dma_start(out=out_d[st * 128 : (st + 1) * 128, :], in_=res)

    nc.compile()
    return nc


def _get_nc():
    if "nc" not in _cache:
        _cache["nc"] = _build()
    return _cache["nc"]


def _ones_bf16():
    import ml_dtypes

    return np.ones((128, NK), ml_dtypes.bfloat16)


def kernel(x, Wq, bq, Wk, bk, Wv, bv, **_):
    from concourse.bass_utils import run_bass_kernel_spmd

    nc = _get_nc()
    x = np.asarray(x, dtype=np.float32)
    common = {
        "wq": np.ascontiguousarray(np.asarray(Wq, np.float32)),
        "wk": np.ascontiguousarray(np.asarray(Wk, np.float32)),
        "wv": np.ascontiguousarray(np.asarray(Wv, np.float32)),
        "bq": np.ascontiguousarray(np.asarray(bq, np.float32).reshape(H, 1)),
        "bk": np.ascontiguousarray(np.asarray(bk, np.float32).reshape(H, 1)),
        "bv": np.ascontiguousarray(np.asarray(bv, np.float32).reshape(H, 1)),
        "ones": _ones_bf16(),
    }
    in_maps = [
        {"xT": np.ascontiguousarray(x[b].T), **common} for b in range(B)
    ]
    res = run_bass_kernel_spmd(nc, in_maps, core_ids=list(range(B)))
    return np.stack([res.results[b]["out"] for b in range(B)])
